# revision 25
# baseline (speedup 1.0000x reference)
"""Trainium2 Bass kernel for nn_AudioNetwork_37512244363307.

Algorithm: the reference applies 4 sequential blocks of
  frame(hop 1024, win 2048) -> rfft -> per-(c,k) linear recurrence over
  frames -> irfft * hann -> overlap-add -> tanh(gain*x)
with identity channel mixing.  The per-channel transfer vectors are ~1%
sparse (<= 24 nonzero of 1025 coeffs), so each block reduces to:
  - forward: per hop-chunk j, a_j(k) = sum_n u_j[n] e^{-2pi i k n/2048}
    for the nonzero k only (matmul against a small DFT basis);
    S[i,k] = a_i(k) + (-1)^k a_{i+1}(k)
  - recurrence o[i] = (S[i] + o[i-1]) * t   (hardware tensor_tensor_scan)
  - synthesis: output chunk j = Ocat[j] @ G where Ocat stacks
    [Re o_j, Im o_j, Re o_{j-1}, Im o_{j-1}] (96 rows) and G folds the
    irfft basis, hann window and overlap-add of the two frames.
Channels x batch are sharded over 8 NeuronCores (8 channels each); the
final sum over channels/blocks is accumulated on-core and reduced on host.
The per-slot transfer/sign tables are broadcast on-chip (tiny DMA) and
the S-build reads forward PSUM directly (no staging copy).
"""
import numpy as np

WS = 2048
STEP = 1024
NCOEF = WS // 2 + 1   # 1025
CPD = 64
NB = 4
B = 4
T = 131072
FRAMES = T // STEP    # 128
FR1 = FRAMES + 1      # 129: leading zero/reset column per batch
NK = 24               # padded nonzero-coeff slots per channel (max nnz 21)
NS = 2 * NK           # 48 slots (Re+Im) per channel
NPAIR = 4
NCORES = 8
CH_PER_CORE = CPD // NCORES  # 8
SUBS = STEP // 128    # 8


def _hann():
    return 0.5 * (1.0 - np.cos(2.0 * np.pi * np.arange(WS) / WS))


def _make_tables(transfers):
    """Host-precomputed DFT/synthesis bases, per (block, channel).

    Returns arrays shaped for direct DMA into SBUF tiles:
      fwdb (NB, CPD, 128, SUBS, 64)  lhsT for forward DFT (slots 48-63 zero)
      synb (NB, CPD, 96, SUBS, 128)  lhsT for synthesis (96 contraction)
      ttsg (128, NB, CPD//2, 2)      per-pair transfer (col 0) / sign (col 1)
    """
    H = _hann()
    n1 = np.arange(STEP)
    fwdb = np.zeros((NB, CPD, 128, SUBS, 64), np.float32)
    synb = np.zeros((NB, CPD, 96, SUBS, 128), np.float32)
    ttsg = np.zeros((128, NB, CPD // 2, 2), np.float32)
    f0basis = np.zeros((CPD, STEP, NS), np.float32)
    f0t2 = np.zeros((CPD, NS), np.float32)
    f0sg = np.zeros((CPD, NS), np.float32)
    for i in range(NB):
        for c in range(CPD):
            t = transfers[i, c]
            ks = np.nonzero(t)[0]
            nk = len(ks)
            if nk > NK:
                raise ValueError("too many nonzero coeffs")
            kpad = np.zeros(NK, np.int64)
            kpad[:nk] = ks
            tpad = np.zeros(NK, np.float32)
            tpad[:nk] = t[ks]
            valid = np.zeros(NK, np.float32)
            valid[:nk] = 1.0
            th = 2.0 * np.pi * kpad[None, :] * n1[:, None] / WS  # (1024, NK)
            cos = np.cos(th) * valid
            sin = np.sin(th) * valid
            fwd = np.zeros((STEP, 64), np.float32)
            fwd[:, 0:NK] = cos
            fwd[:, NK:NS] = -sin
            fwdb[i, c] = fwd.reshape(SUBS, 128, 64).transpose(1, 0, 2)
            sign = np.where(kpad % 2 == 0, 1.0, -1.0).astype(np.float32)
            f = np.where(kpad == 0, 1.0 / WS, 2.0 / WS) * valid
            g1re = f[None, :] * H[:STEP, None] * np.cos(th)
            g1im = -f[None, :] * H[:STEP, None] * np.sin(th)
            g2re = f[None, :] * H[STEP:, None] * sign[None, :] * np.cos(th)
            g2im = -f[None, :] * H[STEP:, None] * sign[None, :] * np.sin(th)
            synth = np.concatenate(
                [g1re.T, g1im.T, g2re.T, g2im.T], axis=0).astype(np.float32)
            synb[i, c] = synth.reshape(96, SUBS, 128)
            # pair row mapping: even channel -> rows 0-47, odd -> rows 64-111
            r0 = 0 if c % 2 == 0 else 64
            t2 = np.concatenate([tpad, tpad])
            s2 = np.concatenate([sign * valid, sign * valid])
            ttsg[r0:r0 + NS, i, c // 2, 0] = t2
            ttsg[r0:r0 + NS, i, c // 2, 1] = s2
            if i == 0:
                f0basis[c] = fwd[:, 0:NS]
                f0t2[c] = t2
                f0sg[c] = s2
    return fwdb, synb, ttsg, f0basis, f0t2, f0sg


def _build_bass(gains, skew=True):
    import concourse.bass as bass
    import concourse.mybir as mybir
    from concourse import bacc, tile

    f32 = mybir.dt.float32
    f16 = mybir.dt.float16
    nc = bacc.Bacc()
    # block 0's forward DFT + scan depend only on the (host-known) input, so
    # its post-scan ocat tables are computed on the host and streamed in.
    oc0 = nc.declare_dram_parameter(
        "oc0", [CH_PER_CORE, 96, B * FR1], f16, isOutput=False)
    fwdblob = nc.declare_dram_parameter(
        "fwdblob", [NB, CH_PER_CORE, 128, SUBS * 64], f16, isOutput=False)
    synblob = nc.declare_dram_parameter(
        "synblob", [NB, CH_PER_CORE, 96, SUBS * 128], f16, isOutput=False)
    ttsg = nc.declare_dram_parameter(
        "ttsg", [128, NB * NPAIR * 2], f32, isOutput=False)
    outq = nc.declare_dram_parameter(
        "outq", [NB, NPAIR, 128, SUBS, B, FRAMES], f16, isOutput=True)

    with tile.TileContext(nc) as tc:
        with (
            tc.tile_pool(name="res", bufs=CH_PER_CORE) as res_pool,
            tc.tile_pool(name="acc", bufs=2) as acc_pool,
            tc.tile_pool(name="basis", bufs=4) as basis_pool,
            tc.tile_pool(name="work", bufs=6) as work_pool,
            tc.tile_pool(name="ttb", bufs=1) as ttb_pool,
            tc.tile_pool(name="fps", bufs=2, space=bass.MemorySpace.PSUM) as fps_pool,
            tc.tile_pool(name="sps", bufs=2, space=bass.MemorySpace.PSUM) as sps_pool,
        ):
            # sub-major layout: tanh writes and fwd matmul reads are
            # contiguous column ranges.
            res = [res_pool.tile([128, SUBS, B, FRAMES], f16, tag="res",
                                 name=f"res{c}")
                   for c in range(CH_PER_CORE)]
            # warmup bridges the initial DMA latency so the PE p-state is
            # ramped when the first real matmul issues; the dummy Tanh pulls
            # the activation-table load off the first real activation.
            warm = work_pool.tile([128, 512], f16, tag="warm", bufs=1)
            warmact = work_pool.tile([128, 16], f16, tag="warmact", bufs=1)
            warmps = fps_pool.tile([128, 512], f32, tag="fps")
            nc.gpsimd.memset(warm[:], 0.0)
            nc.scalar.activation(warmact[:], warm[:, 0:16],
                                 mybir.ActivationFunctionType.Tanh)
            for w in range(8):
                nc.tensor.matmul(warmps[:], warm[:, 0:128], warm[:],
                                 start=(w == 0), stop=(w == 7))

            # per-(block, pair) transfer values land in one tiny tile; the
            # [128, 516] broadcast tiles are built lazily per pair so they
            # never block the first scans.
            tst = ttb_pool.tile([128, NB * NPAIR * 2], f32, tag="tst")
            nc.sync.dma_start(tst[:], ttsg[:])
            zt = ttb_pool.tile([128, B, FR1], f32, tag="zt")
            nc.gpsimd.memset(zt[:], 0.0)

            def front_half(i, p):
                """Paired front: channels (2p, 2p+1) share the fwd PSUM bank
                via column-group tiling, so S-build + scan run once per pair
                on all 128 partitions."""
                c0, c1 = 2 * p, 2 * p + 1
                idx = i * NPAIR + p
                if i == 0:
                    # block 0: ocat comes precomputed from the host; only
                    # load it and the synthesis bases.
                    sb0 = basis_pool.tile([96, SUBS, 128], f16, tag="sb0")
                    sb1 = basis_pool.tile([96, SUBS, 128], f16, tag="sb1")
                    nc.sync.dma_start(
                        sb0[:].rearrange('p s m -> p (s m)'), synblob[i, c0])
                    nc.sync.dma_start(
                        sb1[:].rearrange('p s m -> p (s m)'), synblob[i, c1])
                    ocat0 = work_pool.tile([96, B, FR1], f16, tag="ocat0")
                    ocat1 = work_pool.tile([96, B, FR1], f16, tag="ocat1")
                    nc.gpsimd.dma_start(
                        ocat0[:].rearrange('p b j -> p (b j)'), oc0[c0])
                    nc.gpsimd.dma_start(
                        ocat1[:].rearrange('p b j -> p (b j)'), oc0[c1])
                    return (sb0, ocat0), (sb1, ocat1)
                tt = ttb_pool.tile([128, B, FR1], f32, tag="ttbig",
                                   bufs=4, name=f"tt{i}_{p}")
                nc.vector.scalar_tensor_tensor(
                    tt[:, :, 1:FR1], zt[:, :, 1:FR1],
                    tst[:, 2 * idx:2 * idx + 1], zt[:, :, 1:FR1],
                    mybir.AluOpType.add, mybir.AluOpType.add)
                nc.gpsimd.memset(tt[:, :, 0:1], 0.0)
                fb0 = basis_pool.tile([128, SUBS, 64], f16, tag="fb0")
                fb1 = basis_pool.tile([128, SUBS, 64], f16, tag="fb1")
                nc.sync.dma_start(
                    fb0[:].rearrange('p s m -> p (s m)'), fwdblob[i, c0])
                nc.sync.dma_start(
                    fb1[:].rearrange('p s m -> p (s m)'), fwdblob[i, c1])
                sb0 = basis_pool.tile([96, SUBS, 128], f16, tag="sb0")
                sb1 = basis_pool.tile([96, SUBS, 128], f16, tag="sb1")
                nc.sync.dma_start(
                    sb0[:].rearrange('p s m -> p (s m)'), synblob[i, c0])
                nc.sync.dma_start(
                    sb1[:].rearrange('p s m -> p (s m)'), synblob[i, c1])
                sg = tst[:, 2 * idx + 1:2 * idx + 2]

                fwdps = fps_pool.tile([128, B, FRAMES], f32, tag="fps")
                for s in range(SUBS):
                    nc.tensor.matmul(
                        fwdps[0:64], fb0[:, s, :], res[c0][:, s, :, :],
                        start=(s == 0), stop=(s == SUBS - 1),
                        tile_position=(0, 0), skip_group_check=True)
                    nc.tensor.matmul(
                        fwdps[64:128], fb1[:, s, :], res[c1][:, s, :, :],
                        start=(s == 0), stop=(s == SUBS - 1),
                        tile_position=(0, 64), skip_group_check=True)
                # stile col (b, 1+i) = S[i] = a_i + sign * a_{i+1}, built
                # straight from PSUM; col (b,0) is a reset column (t=0).
                # ISA: stt src0/src1 cannot both be PSUM, so stage a_i into
                # stile first, then add sign * a_{i+1} from PSUM.
                stile = work_pool.tile([128, B, FR1], f32, tag="stile")
                nc.gpsimd.memset(stile[:, :, 0:1], 0.0)
                nc.vector.tensor_copy(
                    stile[:, :, 1:FR1], fwdps[:, :, 0:FRAMES])
                nc.vector.scalar_tensor_tensor(
                    stile[:, :, 1:FRAMES], fwdps[:, :, 1:FRAMES], sg,
                    stile[:, :, 1:FRAMES],
                    mybir.AluOpType.mult, mybir.AluOpType.add)
                # one batched scan: both channels (partition halves) and all
                # b; col (b,0) has t=0 so state resets at batch boundaries.
                opair = work_pool.tile([128, B, FR1], f16, tag="opair")
                nc.vector.tensor_tensor_scan(
                    opair[:].rearrange('p b j -> p (b j)'),
                    stile[:].rearrange('p b j -> p (b j)'),
                    tt[:].rearrange('p b j -> p (b j)'),
                    0.0, mybir.AluOpType.add, mybir.AluOpType.mult)
                ocat0 = work_pool.tile([96, B, FR1], f16, tag="ocat0")
                ocat1 = work_pool.tile([96, B, FR1], f16, tag="ocat1")
                nc.gpsimd.dma_start(
                    ocat0[0:NS, :, 1:FR1], opair[0:NS, :, 1:FR1])
                nc.gpsimd.dma_start(
                    ocat0[NS:96, :, 1:FR1], opair[0:NS, :, 0:FRAMES])
                nc.gpsimd.dma_start(
                    ocat1[0:NS, :, 1:FR1], opair[64:64 + NS, :, 1:FR1])
                nc.gpsimd.dma_start(
                    ocat1[NS:96, :, 1:FR1], opair[64:64 + NS, :, 0:FRAMES])
                return (sb0, ocat0), (sb1, ocat1)

            def back_half(i, c, sb, ocat):
                synrhs = ocat[:, :, 1:FR1]
                for g0, g1 in ((0, 3), (3, 6), (6, 8)):
                    synps = sps_pool.tile([128, 3, B, FRAMES], f32, tag="sps")
                    for h in range(g0, g1):
                        nc.tensor.matmul(
                            synps[:, h - g0], sb[:, h, :], synrhs,
                            start=True, stop=True)
                    nc.scalar.activation(
                        res[c][:, g0:g1, :, :], synps[:, 0:g1 - g0],
                        mybir.ActivationFunctionType.Tanh,
                        scale=float(gains[i]))
                # quarter accumulation: one add per channel pair; the four
                # quarter partials per block are summed on the host.
                if c % 2 == 1:
                    q = c // 2
                    acc = acc_pool.tile([128, SUBS, B, FRAMES], f16,
                                        tag=f"accq{q}", name=f"acc{i}_{q}")
                    nc.vector.tensor_add(acc[:], res[c - 1][:], res[c][:])
                    nc.gpsimd.dma_start(outq[i, q], acc[:])

            from collections import deque
            pend_q = deque()
            for i in range(NB):
                for p in range(CH_PER_CORE // 2):
                    st0, st1 = front_half(i, p)
                    pend_q.append((i, 2 * p, st0[0], st0[1]))
                    pend_q.append((i, 2 * p + 1, st1[0], st1[1]))
                    # drain immediately for the very first pair so the first
                    # activations start as early as possible; steady state
                    # keeps a 2-channel skew between front and back halves.
                    lag = 0 if (i == 0 and p == 0) else (2 if skew else 0)
                    while len(pend_q) > lag:
                        back_half(*pend_q.popleft())
            while pend_q:
                back_half(*pend_q.popleft())
    nc.compile()
    return nc


def _prep_inputs(x, transfers):
    fwdb, synb, ttsg, f0basis, f0t2, f0sg = _make_tables(transfers)
    fwdblob = fwdb.reshape(NB, CPD, 128, SUBS * 64).astype(np.float16)
    synblob = synb.reshape(NB, CPD, 96, SUBS * 128).astype(np.float16)
    # host-side block 0: forward DFT + S-build + scan (f32, then f16)
    x4 = x.reshape(B, CPD, FRAMES, STEP).astype(np.float32)
    a = np.empty((B, CPD, FRAMES, NS), np.float32)
    for c in range(CPD):
        a[:, c] = (x4[:, c].reshape(B * FRAMES, STEP)
                   @ f0basis[c]).reshape(B, FRAMES, NS)
    S = a.copy()
    S[:, :, :FRAMES - 1] += f0sg[None, :, None, :] * a[:, :, 1:]
    o = np.zeros((B, CPD, FRAMES, NS), np.float32)
    state = np.zeros((B, CPD, NS), np.float32)
    for j in range(FRAMES):
        state = (S[:, :, j] + state) * f0t2[None, :, :]
        o[:, :, j] = state
    oc0 = np.zeros((CPD, 96, B, FR1), np.float16)
    oc0[:, 0:NS, :, 1:FR1] = np.transpose(o, (1, 3, 0, 2))
    oc0[:, NS:96, :, 2:FR1] = np.transpose(o[:, :, :FRAMES - 1], (1, 3, 0, 2))
    oc0 = oc0.reshape(CPD, 96, B * FR1)
    in_maps = []
    for core in range(NCORES):
        cl = core * CH_PER_CORE
        ch = cl + CH_PER_CORE
        tts = ttsg[:, :, cl // 2:ch // 2, :].reshape(128, NB * NPAIR * 2)
        in_maps.append({
            "oc0": np.ascontiguousarray(oc0[cl:ch]),
            "fwdblob": np.ascontiguousarray(fwdblob[:, cl:ch]),
            "synblob": np.ascontiguousarray(synblob[:, cl:ch]),
            "ttsg": np.ascontiguousarray(tts),
        })
    return in_maps


def _combine(x, outs, mixer):
    # outs: per-core list of (NB, NPAIR, 128, SUBS, B, FRAMES) partials
    mv = np.exp(mixer - np.max(mixer))
    mv = (mv / mv.sum()).astype(np.float32)
    total = np.zeros((NB, 128, SUBS, B, FRAMES), np.float32)
    for o in outs:
        total += np.asarray(o, np.float32).sum(axis=1)
    mixed = np.einsum('l...,l->...', total, mv[1:])  # (128, SUBS, B, FRAMES)
    y = np.transpose(mixed, (2, 3, 1, 0)).reshape(B, T)  # b, j, s, n'
    y = y + mv[0] * x.sum(axis=1)
    return np.ascontiguousarray(y[:, None, :]).astype(np.float32)


def _kernel_np_fallback(x, transfers, mixer_matrices, gains, mixer):
    H = _hann()
    frames = x.shape[-1] // STEP
    mv = np.exp(mixer - np.max(mixer))
    mv = mv / mv.sum()
    outputs = [x.astype(np.float32)]
    inp = x.astype(np.float32)
    idx = np.arange(frames)[:, None] * STEP + np.arange(WS)[None, :]
    for i in range(NB):
        xm = np.einsum('bct,cd->bdt', inp, mixer_matrices[i])
        xp = np.pad(xm, ((0, 0), (0, 0), (0, WS - STEP)))
        windowed = xp[..., idx]
        spec = np.fft.rfft(windowed, axis=-1)
        Tc = transfers[i].astype(spec.dtype)
        o = np.zeros(spec.shape[:2] + (spec.shape[3],), spec.dtype)
        outspec = np.empty_like(spec)
        for fidx in range(frames):
            o = (spec[:, :, fidx] + o) * Tc[None]
            outspec[:, :, fidx] = o
        wins = np.fft.irfft(outspec, n=WS, axis=-1) * H
        L = (frames - 1) * STEP + WS
        samples = np.zeros(xm.shape[:2] + (L,), np.float32)
        for fidx in range(frames):
            samples[..., fidx * STEP:fidx * STEP + WS] += \
                wins[:, :, fidx].astype(np.float32)
        inp = np.tanh(samples[..., :x.shape[-1]] * gains[i]).astype(np.float32)
        outputs.append(inp)
    result = np.stack(outputs, axis=-1)
    mixed = (result * mv[None, None, None, :]).sum(-1)
    return mixed.sum(axis=1, keepdims=True).astype(np.float32)


def _conforms(x, transfers, mixer_matrices, gains, mixer):
    try:
        if x.shape != (B, CPD, T) or transfers.shape != (NB, CPD, NCOEF):
            return False
        if mixer_matrices.shape != (NB, CPD, CPD) or gains.shape != (NB,):
            return False
        eye = np.eye(CPD, dtype=np.float32)
        if not all(np.array_equal(mixer_matrices[i], eye) for i in range(NB)):
            return False
        if (transfers != 0).sum(axis=-1).max() > NK:
            return False
        # k = WS/2 (Nyquist) term would need a different irfft scale
        if np.any(transfers[:, :, NCOEF - 1] != 0):
            return False
        return True
    except Exception:
        return False


_CACHE = {}


def kernel(**inputs):
    x = np.asarray(inputs["x"], np.float32)
    transfers = np.asarray(inputs["transfers"], np.float32)
    mixer_matrices = np.asarray(inputs["mixer_matrices"], np.float32)
    gains = np.asarray(inputs["gains"], np.float32)
    mixer = np.asarray(inputs["mixer"], np.float32)
    if not _conforms(x, transfers, mixer_matrices, gains, mixer):
        return _kernel_np_fallback(x, transfers, mixer_matrices, gains, mixer)

    from concourse.bass_utils import run_bass_kernel_spmd
    in_maps = _prep_inputs(x, transfers)
    key = gains.tobytes()
    if key not in _CACHE:
        _CACHE[key] = _build_bass(gains)
    nc = _CACHE[key]
    res = run_bass_kernel_spmd(nc, in_maps, list(range(NCORES)))
    outs = [res.results[i]["outq"] for i in range(NCORES)]
    return _combine(x, outs, mixer)


# revision 26
# speedup vs baseline: 1.2322x; 1.2322x over previous
"""Trainium2 Bass kernel for nn_AudioNetwork_37512244363307.

Algorithm: the reference applies 4 sequential blocks of
  frame(hop 1024, win 2048) -> rfft -> per-(c,k) linear recurrence over
  frames -> irfft * hann -> overlap-add -> tanh(gain*x)
with identity channel mixing.  The per-channel transfer vectors are ~1%
sparse (<= 24 nonzero of 1025 coeffs), so each block reduces to:
  - forward: per hop-chunk j, a_j(k) = sum_n u_j[n] e^{-2pi i k n/2048}
    for the nonzero k only (matmul against a small DFT basis);
    S[i,k] = a_i(k) + (-1)^k a_{i+1}(k)
  - recurrence o[i] = (S[i] + o[i-1]) * t   (hardware tensor_tensor_scan)
  - synthesis: output chunk j = Ocat[j] @ G where Ocat stacks
    [Re o_j, Im o_j, Re o_{j-1}, Im o_{j-1}] (96 rows) and G folds the
    irfft basis, hann window and overlap-add of the two frames.
Channels x batch are sharded over 8 NeuronCores (8 channels each); the
final sum over channels/blocks is accumulated on-core and reduced on host.
The per-slot transfer/sign tables are broadcast on-chip (tiny DMA) and
the S-build reads forward PSUM directly (no staging copy).
"""
import numpy as np

WS = 2048
STEP = 1024
NCOEF = WS // 2 + 1   # 1025
CPD = 64
NB = 4
B = 4
T = 131072
FRAMES = T // STEP    # 128
FR1 = FRAMES + 1      # 129: leading zero/reset column per batch
NK = 24               # padded nonzero-coeff slots per channel (max nnz 21)
NS = 2 * NK           # 48 slots (Re+Im) per channel
NPAIR = 4
NCORES = 8
CH_PER_CORE = CPD // NCORES  # 8
SUBS = STEP // 128    # 8


def _hann():
    return 0.5 * (1.0 - np.cos(2.0 * np.pi * np.arange(WS) / WS))


def _make_tables(transfers):
    """Host-precomputed DFT/synthesis bases, per (block, channel).

    Returns arrays shaped for direct DMA into SBUF tiles:
      fwdb (NB, CPD, 128, SUBS, 64)  lhsT for forward DFT (slots 48-63 zero)
      synb (NB, CPD, 96, SUBS, 128)  lhsT for synthesis (96 contraction)
      ttsg (128, NB, CPD//2, 2)      per-pair transfer (col 0) / sign (col 1)
    """
    H = _hann()
    n1 = np.arange(STEP)
    fwdb = np.zeros((NB, CPD, 128, SUBS, 64), np.float32)
    synb = np.zeros((NB, CPD, 96, SUBS, 128), np.float32)
    ttsg = np.zeros((128, NB, CPD // 2, 2), np.float32)
    f0basis = np.zeros((CPD, STEP, NS), np.float32)
    f0t2 = np.zeros((CPD, NS), np.float32)
    f0sg = np.zeros((CPD, NS), np.float32)
    for i in range(NB):
        for c in range(CPD):
            t = transfers[i, c]
            ks = np.nonzero(t)[0]
            nk = len(ks)
            if nk > NK:
                raise ValueError("too many nonzero coeffs")
            kpad = np.zeros(NK, np.int64)
            kpad[:nk] = ks
            tpad = np.zeros(NK, np.float32)
            tpad[:nk] = t[ks]
            valid = np.zeros(NK, np.float32)
            valid[:nk] = 1.0
            th = 2.0 * np.pi * kpad[None, :] * n1[:, None] / WS  # (1024, NK)
            cos = np.cos(th) * valid
            sin = np.sin(th) * valid
            fwd = np.zeros((STEP, 64), np.float32)
            fwd[:, 0:NK] = cos
            fwd[:, NK:NS] = -sin
            fwdb[i, c] = fwd.reshape(SUBS, 128, 64).transpose(1, 0, 2)
            sign = np.where(kpad % 2 == 0, 1.0, -1.0).astype(np.float32)
            f = np.where(kpad == 0, 1.0 / WS, 2.0 / WS) * valid
            g1re = f[None, :] * H[:STEP, None] * np.cos(th)
            g1im = -f[None, :] * H[:STEP, None] * np.sin(th)
            g2re = f[None, :] * H[STEP:, None] * sign[None, :] * np.cos(th)
            g2im = -f[None, :] * H[STEP:, None] * sign[None, :] * np.sin(th)
            synth = np.concatenate(
                [g1re.T, g1im.T, g2re.T, g2im.T], axis=0).astype(np.float32)
            synb[i, c] = synth.reshape(96, SUBS, 128)
            # pair row mapping: even channel -> rows 0-47, odd -> rows 64-111
            r0 = 0 if c % 2 == 0 else 64
            t2 = np.concatenate([tpad, tpad])
            s2 = np.concatenate([sign * valid, sign * valid])
            ttsg[r0:r0 + NS, i, c // 2, 0] = t2
            ttsg[r0:r0 + NS, i, c // 2, 1] = s2
            if i == 0:
                f0basis[c] = fwd[:, 0:NS]
                f0t2[c] = t2
                f0sg[c] = s2
    return fwdb, synb, ttsg, f0basis, f0t2, f0sg


def _build_bass(gains, skew=True):
    import concourse.bass as bass
    import concourse.mybir as mybir
    from concourse import bacc, tile

    f32 = mybir.dt.float32
    f16 = mybir.dt.float16
    nc = bacc.Bacc()
    # block 0's forward DFT + scan depend only on the (host-known) input, so
    # its post-scan ocat tables are computed on the host and streamed in.
    oc0 = nc.declare_dram_parameter(
        "oc0", [CH_PER_CORE, 96, B * FR1], f16, isOutput=False)
    fwdblob = nc.declare_dram_parameter(
        "fwdblob", [NB, CH_PER_CORE, 128, SUBS * 64], f16, isOutput=False)
    synblob = nc.declare_dram_parameter(
        "synblob", [NB, CH_PER_CORE, 96, SUBS * 128], f16, isOutput=False)
    ttsg = nc.declare_dram_parameter(
        "ttsg", [128, NB * NPAIR * 2], f32, isOutput=False)
    outq = nc.declare_dram_parameter(
        "outq", [NB, NPAIR, 128, SUBS, B, FRAMES], f16, isOutput=True)

    with tile.TileContext(nc) as tc:
        with (
            tc.tile_pool(name="res", bufs=CH_PER_CORE) as res_pool,
            tc.tile_pool(name="acc", bufs=2) as acc_pool,
            tc.tile_pool(name="basis", bufs=4) as basis_pool,
            tc.tile_pool(name="work", bufs=6) as work_pool,
            tc.tile_pool(name="ttb", bufs=1) as ttb_pool,
            tc.tile_pool(name="fps", bufs=2, space=bass.MemorySpace.PSUM) as fps_pool,
            tc.tile_pool(name="sps", bufs=2, space=bass.MemorySpace.PSUM) as sps_pool,
        ):
            # sub-major layout: tanh writes and fwd matmul reads are
            # contiguous column ranges.
            res = [res_pool.tile([128, SUBS, B, FRAMES], f16, tag="res",
                                 name=f"res{c}")
                   for c in range(CH_PER_CORE)]
            # warmup bridges the initial DMA latency so the PE p-state is
            # ramped when the first real matmul issues; the dummy Tanh pulls
            # the activation-table load off the first real activation.
            warm = work_pool.tile([128, 512], f16, tag="warm", bufs=1)
            warmact = work_pool.tile([128, 16], f16, tag="warmact", bufs=1)
            warmps = fps_pool.tile([128, 512], f32, tag="fps")
            nc.gpsimd.memset(warm[:], 0.0)
            nc.scalar.activation(warmact[:], warm[:, 0:16],
                                 mybir.ActivationFunctionType.Tanh)
            for w in range(8):
                nc.tensor.matmul(warmps[:], warm[:, 0:128], warm[:],
                                 start=(w == 0), stop=(w == 7))

            # per-(block, pair) transfer values land in one tiny tile; the
            # [128, 516] broadcast tiles are built lazily per pair so they
            # never block the first scans.
            tst = ttb_pool.tile([128, NB * NPAIR * 2], f32, tag="tst")
            nc.sync.dma_start(tst[:], ttsg[:])
            zt = ttb_pool.tile([128, B, FR1], f32, tag="zt")
            nc.gpsimd.memset(zt[:], 0.0)

            def front_half(i, p):
                """Paired front: channels (2p, 2p+1) share the fwd PSUM bank
                via column-group tiling, so S-build + scan run once per pair
                on all 128 partitions."""
                c0, c1 = 2 * p, 2 * p + 1
                idx = i * NPAIR + p
                if i == 0:
                    # block 0: ocat comes precomputed from the host; only
                    # load it and the synthesis bases.
                    sb0 = basis_pool.tile([96, SUBS, 128], f16, tag="sb0")
                    sb1 = basis_pool.tile([96, SUBS, 128], f16, tag="sb1")
                    nc.sync.dma_start(
                        sb0[:].rearrange('p s m -> p (s m)'), synblob[i, c0])
                    nc.sync.dma_start(
                        sb1[:].rearrange('p s m -> p (s m)'), synblob[i, c1])
                    ocat0 = work_pool.tile([96, B, FR1], f16, tag="ocat0")
                    ocat1 = work_pool.tile([96, B, FR1], f16, tag="ocat1")
                    nc.gpsimd.dma_start(
                        ocat0[:].rearrange('p b j -> p (b j)'), oc0[c0])
                    nc.gpsimd.dma_start(
                        ocat1[:].rearrange('p b j -> p (b j)'), oc0[c1])
                    return (sb0, ocat0), (sb1, ocat1)
                tt = ttb_pool.tile([128, B, FR1], f32, tag="ttbig",
                                   bufs=4, name=f"tt{i}_{p}")
                nc.vector.scalar_tensor_tensor(
                    tt[:, :, 1:FR1], zt[:, :, 1:FR1],
                    tst[:, 2 * idx:2 * idx + 1], zt[:, :, 1:FR1],
                    mybir.AluOpType.add, mybir.AluOpType.add)
                nc.gpsimd.memset(tt[:, :, 0:1], 0.0)
                fb0 = basis_pool.tile([128, SUBS, 64], f16, tag="fb0")
                fb1 = basis_pool.tile([128, SUBS, 64], f16, tag="fb1")
                nc.sync.dma_start(
                    fb0[:].rearrange('p s m -> p (s m)'), fwdblob[i, c0])
                nc.sync.dma_start(
                    fb1[:].rearrange('p s m -> p (s m)'), fwdblob[i, c1])
                sb0 = basis_pool.tile([96, SUBS, 128], f16, tag="sb0")
                sb1 = basis_pool.tile([96, SUBS, 128], f16, tag="sb1")
                nc.sync.dma_start(
                    sb0[:].rearrange('p s m -> p (s m)'), synblob[i, c0])
                nc.sync.dma_start(
                    sb1[:].rearrange('p s m -> p (s m)'), synblob[i, c1])
                sg = tst[:, 2 * idx + 1:2 * idx + 2]

                fwdps = fps_pool.tile([128, B, FRAMES], f32, tag="fps")
                for s in range(SUBS):
                    nc.tensor.matmul(
                        fwdps[0:64], fb0[:, s, :], res[c0][:, s, :, :],
                        start=(s == 0), stop=(s == SUBS - 1),
                        tile_position=(0, 0), skip_group_check=True)
                    nc.tensor.matmul(
                        fwdps[64:128], fb1[:, s, :], res[c1][:, s, :, :],
                        start=(s == 0), stop=(s == SUBS - 1),
                        tile_position=(0, 64), skip_group_check=True)
                # stile col (b, 1+i) = S[i] = a_i + sign * a_{i+1}, built
                # straight from PSUM; col (b,0) is a reset column (t=0).
                # ISA: stt src0/src1 cannot both be PSUM, so stage a_i into
                # stile first, then add sign * a_{i+1} from PSUM.
                stile = work_pool.tile([128, B, FR1], f32, tag="stile")
                nc.gpsimd.memset(stile[:, :, 0:1], 0.0)
                nc.vector.tensor_copy(
                    stile[:, :, 1:FR1], fwdps[:, :, 0:FRAMES])
                nc.vector.scalar_tensor_tensor(
                    stile[:, :, 1:FRAMES], fwdps[:, :, 1:FRAMES], sg,
                    stile[:, :, 1:FRAMES],
                    mybir.AluOpType.mult, mybir.AluOpType.add)
                # one batched scan: both channels (partition halves) and all
                # b; col (b,0) has t=0 so state resets at batch boundaries.
                opair = work_pool.tile([128, B, FR1], f16, tag="opair")
                nc.vector.tensor_tensor_scan(
                    opair[:].rearrange('p b j -> p (b j)'),
                    stile[:].rearrange('p b j -> p (b j)'),
                    tt[:].rearrange('p b j -> p (b j)'),
                    0.0, mybir.AluOpType.add, mybir.AluOpType.mult)
                ocat0 = work_pool.tile([96, B, FR1], f16, tag="ocat0")
                ocat1 = work_pool.tile([96, B, FR1], f16, tag="ocat1")
                nc.gpsimd.dma_start(
                    ocat0[0:NS, :, 1:FR1], opair[0:NS, :, 1:FR1])
                nc.gpsimd.dma_start(
                    ocat0[NS:96, :, 1:FR1], opair[0:NS, :, 0:FRAMES])
                nc.gpsimd.dma_start(
                    ocat1[0:NS, :, 1:FR1], opair[64:64 + NS, :, 1:FR1])
                nc.gpsimd.dma_start(
                    ocat1[NS:96, :, 1:FR1], opair[64:64 + NS, :, 0:FRAMES])
                return (sb0, ocat0), (sb1, ocat1)

            def back_half(i, c, sb, ocat):
                synrhs = ocat[:, :, 1:FR1]
                for g0, g1 in ((0, 3), (3, 6), (6, 8)):
                    synps = sps_pool.tile([128, 3, B, FRAMES], f32, tag="sps")
                    for h in range(g0, g1):
                        nc.tensor.matmul(
                            synps[:, h - g0], sb[:, h, :], synrhs,
                            start=True, stop=True)
                    nc.scalar.activation(
                        res[c][:, g0:g1, :, :], synps[:, 0:g1 - g0],
                        mybir.ActivationFunctionType.Tanh,
                        scale=float(gains[i]))
                # quarter accumulation: one add per channel pair; the four
                # quarter partials per block are summed on the host.  Adds
                # and output DMAs are split by activation group so the tail
                # drains while later groups still compute.
                if c % 2 == 1:
                    q = c // 2
                    acc = acc_pool.tile([128, SUBS, B, FRAMES], f16,
                                        tag=f"accq{q}", name=f"acc{i}_{q}")
                    for g0, g1 in ((0, 3), (3, 6), (6, 8)):
                        nc.vector.tensor_add(
                            acc[:, g0:g1], res[c - 1][:, g0:g1],
                            res[c][:, g0:g1])
                        nc.gpsimd.dma_start(
                            outq[i, q, :, g0:g1], acc[:, g0:g1])

            from collections import deque
            pend_q = deque()
            for i in range(NB):
                for p in range(CH_PER_CORE // 2):
                    st0, st1 = front_half(i, p)
                    pend_q.append((i, 2 * p, st0[0], st0[1]))
                    pend_q.append((i, 2 * p + 1, st1[0], st1[1]))
                    # drain immediately for the very first pair so the first
                    # activations start as early as possible; steady state
                    # keeps a 2-channel skew between front and back halves.
                    lag = 0 if (i == 0 and p == 0) else (2 if skew else 0)
                    while len(pend_q) > lag:
                        back_half(*pend_q.popleft())
            while pend_q:
                back_half(*pend_q.popleft())
    nc.compile()
    return nc


def _prep_inputs(x, transfers):
    fwdb, synb, ttsg, f0basis, f0t2, f0sg = _make_tables(transfers)
    fwdblob = fwdb.reshape(NB, CPD, 128, SUBS * 64).astype(np.float16)
    synblob = synb.reshape(NB, CPD, 96, SUBS * 128).astype(np.float16)
    # host-side block 0: forward DFT + S-build + scan (f32, then f16)
    x4 = x.reshape(B, CPD, FRAMES, STEP).astype(np.float32)
    a = np.empty((B, CPD, FRAMES, NS), np.float32)
    for c in range(CPD):
        a[:, c] = (x4[:, c].reshape(B * FRAMES, STEP)
                   @ f0basis[c]).reshape(B, FRAMES, NS)
    S = a.copy()
    S[:, :, :FRAMES - 1] += f0sg[None, :, None, :] * a[:, :, 1:]
    o = np.zeros((B, CPD, FRAMES, NS), np.float32)
    state = np.zeros((B, CPD, NS), np.float32)
    for j in range(FRAMES):
        state = (S[:, :, j] + state) * f0t2[None, :, :]
        o[:, :, j] = state
    oc0 = np.zeros((CPD, 96, B, FR1), np.float16)
    oc0[:, 0:NS, :, 1:FR1] = np.transpose(o, (1, 3, 0, 2))
    oc0[:, NS:96, :, 2:FR1] = np.transpose(o[:, :, :FRAMES - 1], (1, 3, 0, 2))
    oc0 = oc0.reshape(CPD, 96, B * FR1)
    in_maps = []
    for core in range(NCORES):
        cl = core * CH_PER_CORE
        ch = cl + CH_PER_CORE
        tts = ttsg[:, :, cl // 2:ch // 2, :].reshape(128, NB * NPAIR * 2)
        in_maps.append({
            "oc0": np.ascontiguousarray(oc0[cl:ch]),
            "fwdblob": np.ascontiguousarray(fwdblob[:, cl:ch]),
            "synblob": np.ascontiguousarray(synblob[:, cl:ch]),
            "ttsg": np.ascontiguousarray(tts),
        })
    return in_maps


def _combine(x, outs, mixer):
    # outs: per-core list of (NB, NPAIR, 128, SUBS, B, FRAMES) partials
    mv = np.exp(mixer - np.max(mixer))
    mv = (mv / mv.sum()).astype(np.float32)
    total = np.zeros((NB, 128, SUBS, B, FRAMES), np.float32)
    for o in outs:
        total += np.asarray(o, np.float32).sum(axis=1)
    mixed = np.einsum('l...,l->...', total, mv[1:])  # (128, SUBS, B, FRAMES)
    y = np.transpose(mixed, (2, 3, 1, 0)).reshape(B, T)  # b, j, s, n'
    y = y + mv[0] * x.sum(axis=1)
    return np.ascontiguousarray(y[:, None, :]).astype(np.float32)


def _kernel_np_fallback(x, transfers, mixer_matrices, gains, mixer):
    H = _hann()
    frames = x.shape[-1] // STEP
    mv = np.exp(mixer - np.max(mixer))
    mv = mv / mv.sum()
    outputs = [x.astype(np.float32)]
    inp = x.astype(np.float32)
    idx = np.arange(frames)[:, None] * STEP + np.arange(WS)[None, :]
    for i in range(NB):
        xm = np.einsum('bct,cd->bdt', inp, mixer_matrices[i])
        xp = np.pad(xm, ((0, 0), (0, 0), (0, WS - STEP)))
        windowed = xp[..., idx]
        spec = np.fft.rfft(windowed, axis=-1)
        Tc = transfers[i].astype(spec.dtype)
        o = np.zeros(spec.shape[:2] + (spec.shape[3],), spec.dtype)
        outspec = np.empty_like(spec)
        for fidx in range(frames):
            o = (spec[:, :, fidx] + o) * Tc[None]
            outspec[:, :, fidx] = o
        wins = np.fft.irfft(outspec, n=WS, axis=-1) * H
        L = (frames - 1) * STEP + WS
        samples = np.zeros(xm.shape[:2] + (L,), np.float32)
        for fidx in range(frames):
            samples[..., fidx * STEP:fidx * STEP + WS] += \
                wins[:, :, fidx].astype(np.float32)
        inp = np.tanh(samples[..., :x.shape[-1]] * gains[i]).astype(np.float32)
        outputs.append(inp)
    result = np.stack(outputs, axis=-1)
    mixed = (result * mv[None, None, None, :]).sum(-1)
    return mixed.sum(axis=1, keepdims=True).astype(np.float32)


def _conforms(x, transfers, mixer_matrices, gains, mixer):
    try:
        if x.shape != (B, CPD, T) or transfers.shape != (NB, CPD, NCOEF):
            return False
        if mixer_matrices.shape != (NB, CPD, CPD) or gains.shape != (NB,):
            return False
        eye = np.eye(CPD, dtype=np.float32)
        if not all(np.array_equal(mixer_matrices[i], eye) for i in range(NB)):
            return False
        if (transfers != 0).sum(axis=-1).max() > NK:
            return False
        # k = WS/2 (Nyquist) term would need a different irfft scale
        if np.any(transfers[:, :, NCOEF - 1] != 0):
            return False
        return True
    except Exception:
        return False


_CACHE = {}


def kernel(**inputs):
    x = np.asarray(inputs["x"], np.float32)
    transfers = np.asarray(inputs["transfers"], np.float32)
    mixer_matrices = np.asarray(inputs["mixer_matrices"], np.float32)
    gains = np.asarray(inputs["gains"], np.float32)
    mixer = np.asarray(inputs["mixer"], np.float32)
    if not _conforms(x, transfers, mixer_matrices, gains, mixer):
        return _kernel_np_fallback(x, transfers, mixer_matrices, gains, mixer)

    from concourse.bass_utils import run_bass_kernel_spmd
    in_maps = _prep_inputs(x, transfers)
    key = gains.tobytes()
    if key not in _CACHE:
        _CACHE[key] = _build_bass(gains)
    nc = _CACHE[key]
    res = run_bass_kernel_spmd(nc, in_maps, list(range(NCORES)))
    outs = [res.results[i]["outq"] for i in range(NCORES)]
    return _combine(x, outs, mixer)
